# revision 47
# baseline (speedup 1.0000x reference)
"""Trainium2 Bass kernel for nn_DistLayer (segment-mean pooling + fc + BatchNorm + ReLU).

Contract: kernel(**inputs) takes FULL unsharded numpy inputs and returns the
FULL [131072, 256] float32 output. Internally shards rows across 8 NeuronCores.

Math (reference):
    pooled_atom = segment_mean(x[:, :128], atom_idx)[atom_idx]
    pooled_ele  = segment_mean(x[:, 128:256], atom_idx)[ele_idx]
    h = concat([x_atom, pooled_atom, x_ele, pooled_ele, x_dist]) @ W1 + b1
    out = relu(batchnorm(h))                    (training-mode batch stats)

Device decomposition (per core, h kept feature-major "h^T" [256, rows]):
  atom_idx is sorted, so each core's rows touch a contiguous ~512-segment
  window. The local segment-mean table is kept window-relative in DRAM.
  Cross-core exchange is a small AllGather of the f16 windows (E-half for the
  random ele gathers + the two boundary rows of the A-half); the <=7 segments
  split across core boundaries are patched afterwards with tiny indirect DMAs.

  Phase A: one-hot segment sums (one-hots built on-chip from iota==a_loc),
           scaled by 1/global_count at PSUM flush, scatter-added into the
           window table; pack + AllGather the window.
  Phase B: h^T x-part = Wx^T x^T streamed in f16 (overlaps the AllGather).
  Phase C: pooled parts: ele via batched dma_gather from the gathered table,
           atom via ZcT = (Ma_win @ W_pa) expand-matmul with shipped onehot^T;
           fused SBUF+PSUM add & per-feature sum stat in one DVE
           tensor_tensor_reduce; square stat via Act accum.
  Tail:    BN stats AllGather (16KB), affine+relu streamed out in f16
           (host casts to f32).
"""

import os
from contextlib import ExitStack

import numpy as np

import concourse.bass as bass
import concourse.tile as tile
from concourse import bacc, mybir
from concourse.bass_utils import run_bass_kernel_spmd

LAST_NC = None  # most recent built program (for cost-model timing in test.py)

F32 = mybir.dt.float32
F16 = mybir.dt.float16
I16 = mybir.dt.int16
I32 = mybir.dt.int32

N_AE = 128
N_DE = 128
NUM_SEG = 4096
EPS = 1e-5
D_IN = 384            # x feature dim
D_OUT = 256           # output feature dim
BLK = 512             # rows per block
TPB = BLK // 128      # row-tiles per block
PAD = 16              # table front pad


def _wrap16(idx, reps=8):
    """dma_gather index layout: idx i at [i%16, i//16], replicated to 128 rows."""
    n = idx.shape[0]
    w = idx.reshape(n // 16, 16).T.astype(np.int16)   # [16, n/16]
    return np.tile(w, (reps, 1))                       # [128, n/16]


def build_program(n_cores, rpc, w_blk, gsz, W, gS, LT):
    """Build the core-uniform bass program.

    gS[g]/LT[g]: static 128-aligned window-row base and row count of group
    g's private scatter table (uniform across cores).
    """
    nblk = rpc // BLK
    ngrp = nblk // gsz
    ntile = nblk // 2          # 2-block tiles
    WROW = W + 8               # window rows: W E-rows, 2 A-boundary rows, 6 zero
    NCH = (W + 127) // 128

    nc = bacc.Bacc("TRN2", target_bir_lowering=False, debug=False,
                   num_devices=n_cores)

    # ---- I/O tensors (per-core) ----
    PW = TPB * 2 * N_AE  # p1 per-block cols: 4 x_ae row-tiles
    d_xt = nc.dram_tensor("xt", [ntile, 128, 2 * 3 * BLK], F16, kind="ExternalInput").ap()
    d_p1 = nc.dram_tensor("p1", [ntile, 128, 2 * PW], F16, kind="ExternalInput").ap()
    d_alv = nc.dram_tensor("alv", [128, nblk * TPB], F32, kind="ExternalInput").ap()
    d_oht = nc.dram_tensor("oht", [ntile, 128, 2 * BLK], F16, kind="ExternalInput").ap()
    d_iot = nc.dram_tensor("iot", [128, w_blk], F16, kind="ExternalInput").ap()
    d_egi = nc.dram_tensor("egi", [128, rpc // 16], I16, kind="ExternalInput").ap()
    d_awi = nc.dram_tensor("awi", [128, 8 * ngrp], I16, kind="ExternalInput").ap()
    d_scl = nc.dram_tensor("scl", [w_blk, ngrp], F32, kind="ExternalInput").ap()
    d_sof = nc.dram_tensor("sof", [w_blk, ngrp], I32, kind="ExternalInput").ap()
    d_sof2 = nc.dram_tensor("sof2", [w_blk, 1], I32, kind="ExternalInput").ap()
    d_wfo = nc.dram_tensor("wfo", [32, 1], I32, kind="ExternalInput").ap()
    d_afo32 = nc.dram_tensor("afo32", [32, 1], I32, kind="ExternalInput").ap()
    d_lcv = nc.dram_tensor("lcv", [128, 8], F16, kind="ExternalInput").ap()
    d_cntv = nc.dram_tensor("cntv", [128, 8], F16, kind="ExternalInput").ap()
    d_hev = nc.dram_tensor("hev", [128, n_cores * (W + 8) // 128], F16,
                           kind="ExternalInput").ap()
    d_wx = nc.dram_tensor("wx", [128, 3 * D_OUT], F16, kind="ExternalInput").ap()
    d_wpa = nc.dram_tensor("wpa", [N_AE, D_OUT], F16, kind="ExternalInput").ap()
    d_wpe = nc.dram_tensor("wpe", [N_AE, D_OUT], F16, kind="ExternalInput").ap()
    d_gb = nc.dram_tensor("gb", [128, 4], F32, kind="ExternalInput").ap()

    d_out = nc.dram_tensor("out", [D_OUT, rpc], F16, kind="ExternalOutput").ap()

    groups = [list(range(n_cores))]

    with tile.TileContext(nc) as tc, ExitStack() as ctx:
        const = ctx.enter_context(tc.tile_pool(name="const", bufs=1))
        store = ctx.enter_context(tc.tile_pool(name="store", bufs=1))
        strm = ctx.enter_context(tc.tile_pool(name="strm", bufs=2))
        pA = ctx.enter_context(tc.tile_pool(name="pA", bufs=10))
        pH = ctx.enter_context(tc.tile_pool(name="pH", bufs=6))
        pB = ctx.enter_context(tc.tile_pool(name="pB", bufs=3))
        pO = ctx.enter_context(tc.tile_pool(name="pO", bufs=4))
        psm = ctx.enter_context(tc.tile_pool(name="psm", bufs=2, space="PSUM"))
        pss = ctx.enter_context(tc.tile_pool(name="pss", bufs=2, space="PSUM"))
        psz = ctx.enter_context(tc.tile_pool(name="psz", bufs=1, space="PSUM"))
        dram = ctx.enter_context(tc.tile_pool(name="dram", bufs=1, space="DRAM"))

        # internal DRAM (each group table gets a 128-row junk tail for
        # the parked dummy scatter lanes)
        gtabs = [dram.tile([LT[g] + 128, D_OUT], F32, name=f"gtab{g}")
                 for g in range(ngrp)]                   # per-group partial tables
        mtab = dram.tile([8, D_OUT], F32)                # last-segment mirror row
        ptable = dram.tile([WROW, 128], F16)             # packed AllGather input
        gtable = dram.tile([n_cores * WROW, 128], F16, addr_space="Shared")
        wtab = dram.tile([n_cores * WROW, 128], F16)     # fixable copy of gtable
        atab = dram.tile([NCH * 128, 128], F16)          # fixed A-half means, f16
        statin = dram.tile([128, 4], F32)
        statout = dram.tile([n_cores, 128, 4], F32, addr_space="Shared")

        # ---- constants in SBUF ----
        wx = const.tile([128, 3 * D_OUT], F16)
        nc.sync.dma_start(wx[:], d_wx[:])
        wpa = const.tile([128, D_OUT], F16)
        nc.sync.dma_start(wpa[:], d_wpa[:])
        wpe = const.tile([128, D_OUT], F16)
        nc.sync.dma_start(wpe[:], d_wpe[:])
        gb = const.tile([128, 4], F32)
        nc.sync.dma_start(gb[:], d_gb[:])
        iot = const.tile([128, w_blk], F16)
        nc.sync.dma_start(iot[:], d_iot[:])
        alv = const.tile([128, nblk * TPB], F32)
        nc.sync.dma_start(alv[:], d_alv[:])
        egi = const.tile([128, rpc // 16], I16)
        nc.sync.dma_start(egi[:], d_egi[:])
        awi = const.tile([128, 8 * ngrp], I16)
        nc.sync.dma_start(awi[:], d_awi[:])
        scl = const.tile([w_blk, ngrp], F32)
        nc.sync.dma_start(scl[:], d_scl[:])
        sof = const.tile([w_blk, ngrp], I32)
        nc.sync.dma_start(sof[:], d_sof[:])
        sof2 = const.tile([w_blk, 1], I32)
        nc.sync.dma_start(sof2[:], d_sof2[:])
        wfo = const.tile([32, 1], I32)
        nc.sync.dma_start(wfo[:], d_wfo[:])
        afo32 = const.tile([32, 1], I32)
        nc.sync.dma_start(afo32[:], d_afo32[:])
        lcv = const.tile([128, 8], F16)
        nc.sync.dma_start(lcv[:], d_lcv[:])
        cntv = const.tile([128, 8], F16)
        nc.sync.dma_start(cntv[:], d_cntv[:])
        NW = n_cores * WROW // 128
        hev = const.tile([128, NW], F16)
        nc.sync.dma_start(hev[:], d_hev[:])

        # persistent h^T store: 2 feature-chunks of [128, rpc] f16
        hsb = [store.tile([128, rpc], F16, name=f"hsb{m}", tag=f"hsb{m}")
               for m in range(2)]
        sums = store.tile([128, 4 * ntile], F32)  # [h^2 sums | x_dist colsums]

        # ---- zero the per-group tables ----
        nzc = max(LT) // 128 + 1
        zt = const.tile([128, nzc * D_OUT], F32)
        nc.vector.memset(zt[:], 0.0)
        for g in range(ngrp):
            zc = LT[g] // 128 + 1
            nc.sync.dma_start(gtabs[g][:].rearrange("(c p) f -> p c f", p=128),
                              zt[:, 0:zc * D_OUT]
                              .rearrange("p (c f) -> p c f", c=zc))
        nc.sync.dma_start(mtab[:], zt[0:8, 0:D_OUT])

        # ---- Phase A: one-hot segment sums -> scatter into window table ----
        for g in range(ngrp):
            seg = pss.tile([w_blk, D_OUT], F32, name="seg", tag="seg")
            for j in range(gsz):
                b = g * gsz + j
                bt, j2 = b // 2, b % 2
                if j2 == 0:
                    p1t = pA.tile([128, 2 * PW], F16, name="p1t", tag="p1t")
                    nc.sync.dma_start(p1t[:], d_p1[bt])
                ohb = pH.tile([128, TPB * w_blk], F16, name="ohb", tag="ohb")
                for t in range(TPB):
                    nc.vector.tensor_scalar(
                        out=ohb[:, w_blk * t:w_blk * (t + 1)],
                        in0=iot[:],
                        scalar1=alv[:, TPB * b + t:TPB * b + t + 1],
                        scalar2=None,
                        op0=mybir.AluOpType.is_equal)
                for t in range(TPB):
                    nc.tensor.matmul(seg[:],
                                     ohb[:, w_blk * t:w_blk * (t + 1)],
                                     p1t[:, PW * j2 + 2 * N_AE * t:
                                         PW * j2 + 2 * N_AE * (t + 1)],
                                     start=(j == 0 and t == 0),
                                     stop=(j == gsz - 1 and t == TPB - 1))
            ssb = strm.tile([w_blk, D_OUT], F32, name="ssb", tag="ssb")
            nc.scalar.activation(ssb[:], seg[:],
                                 mybir.ActivationFunctionType.Identity,
                                 bias=0.0, scale=scl[:, g:g + 1])
            nc.gpsimd.indirect_dma_start(
                out=gtabs[g][0:LT[g] + 8],
                out_offset=bass.IndirectOffsetOnAxis(ap=sof[:, g:g + 1], axis=0),
                in_=ssb[:],
                in_offset=None,
                compute_op=mybir.AluOpType.add)
            if g == ngrp - 1:
                # mirror the last segment's partial into mtab row 0
                nc.gpsimd.indirect_dma_start(
                    out=mtab[:],
                    out_offset=bass.IndirectOffsetOnAxis(ap=sof2[:], axis=0),
                    in_=ssb[:],
                    in_offset=None,
                    compute_op=mybir.AluOpType.add)

        # ---- merge group tables in SBUF (f32), cast, pack & AllGather ----
        mk32 = const.tile([128, NCH * D_OUT], F32)
        nc.vector.memset(mk32[:], 0.0)
        for g in range(ngrp):
            tmp = strm.tile([128, (max(LT) // 128) * D_OUT], F32,
                            name="tmp", tag="tmp")
            zc = LT[g] // 128
            nc.sync.dma_start(tmp[:, 0:zc * D_OUT]
                              .rearrange("p (c f) -> p c f", c=zc),
                              gtabs[g][0:LT[g], :]
                              .rearrange("(c p) f -> p c f", p=128))
            c0 = gS[g] // 128
            sl = mk32[:, c0 * D_OUT:(c0 + zc) * D_OUT]
            nc.vector.tensor_tensor(out=sl, in0=sl, in1=tmp[:, 0:zc * D_OUT],
                                    op=mybir.AluOpType.add)
        mk16 = const.tile([128, NCH * D_OUT], F16)
        nc.scalar.copy(mk16[:], mk32[:])
        mkv = mk16[:].rearrange("p (c f) -> p c f", c=NCH)
        WF = W // 128  # full chunks
        if WF:
            nc.scalar.dma_start(
                ptable[0:WF * 128, :].rearrange("(c p) f -> p c f", p=128),
                mkv[:, 0:WF, N_AE:2 * N_AE])
        if W % 128:
            nc.scalar.dma_start(ptable[WF * 128:W, :],
                              mk16[0:W % 128,
                                   WF * D_OUT + N_AE:WF * D_OUT + 2 * N_AE])
        # boundary payload rows: W: A-first, W+1: A-last, W+2: E-first,
        # W+3: E-last, W+4..W+7: zeros. The first segment is window row 0
        # (mk16 partition 0, chunk 0); the last segment's partial sits in mtab.
        m32 = const.tile([1, D_OUT], F32)
        nc.sync.dma_start(m32[:], mtab[0:1, :])
        m16 = const.tile([1, D_OUT], F16)
        nc.scalar.copy(m16[:], m32[:])
        nc.scalar.dma_start(ptable[W:W + 1, :], mk16[0:1, 0:N_AE])
        nc.sync.dma_start(ptable[W + 1:W + 2, :], m16[:, 0:N_AE])
        nc.scalar.dma_start(ptable[W + 2:W + 3, :], mk16[0:1, N_AE:2 * N_AE])
        nc.sync.dma_start(ptable[W + 3:W + 4, :], m16[:, N_AE:2 * N_AE])
        z16 = const.tile([8, N_AE], F16)
        nc.vector.memset(z16[:], 0.0)
        nc.sync.dma_start(ptable[W + 4:W + 8, :], z16[0:4, :])
        # store the merged A-half means for the pooled-atom expand path
        nc.sync.dma_start(
            atab[:].rearrange("(c p) f -> p c f", p=128),
            mkv[:, :, 0:N_AE])
        nc.gpsimd.collective_compute(
            "AllGather", mybir.AluOpType.bypass, replica_groups=groups,
            ins=[ptable.opt()], outs=[gtable.opt()])

        # ---- Phase B: h^T x-part (overlaps the AllGather) ----
        for bt in range(ntile):
            xtt = pB.tile([128, 2 * 3 * BLK], F16, name="xtt", tag="xtt")
            nc.sync.dma_start(xtt[:], d_xt[bt])
            for j in range(2):
                nc.vector.tensor_reduce(
                    out=sums[:, 32 + 2 * bt + j:32 + 2 * bt + j + 1],
                    in_=xtt[:, 3 * BLK * j + 2 * BLK:3 * BLK * (j + 1)],
                    axis=mybir.AxisListType.X, op=mybir.AluOpType.add)
            for m in range(2):
                for j in range(2):
                    b = 2 * bt + j
                    pb = psm.tile([128, BLK], F32, name=f"pb{m}", tag=f"pp{m}")
                    for k in range(3):
                        nc.tensor.matmul(
                            pb[:],
                            wx[:, D_OUT * k + 128 * m:D_OUT * k + 128 * (m + 1)],
                            xtt[:, 3 * BLK * j + BLK * k:3 * BLK * j + BLK * (k + 1)],
                            start=(k == 0), stop=(k == 2))
                    nc.scalar.copy(hsb[m][:, BLK * b:BLK * (b + 1)], pb[:])

        # prefetch onehot^T tiles for phase C (issued from the idle DVE queue)
        ohtp = ctx.enter_context(tc.tile_pool(name="ohtp", bufs=6))
        ohtt = []
        for bt in range(ntile):
            o = ohtp.tile([128, 2 * BLK], F16, name="ohtt", tag="ohtt")
            nc.sync.dma_start(o[:], d_oht[bt])
            ohtt.append(o)

        # ---- boundary fix on wtab (fixable copy) + local A-half ----
        nc.scalar.dma_start(wtab[:], gtable[:])
        # all cores' boundary payload rows [8 cores x 4 rows]
        t32 = const.tile([32, 128], F16)
        nc.sync.dma_start(
            t32[:],
            gtable[:].rearrange("(c w) f -> c w f", w=WROW)[:, W:W + 4, :])
        # E pair fix: wtab[L] += right core's E-first, wtab[R] += left's E-last
        nc.gpsimd.indirect_dma_start(
            out=wtab[:],
            out_offset=bass.IndirectOffsetOnAxis(ap=wfo[:], axis=0),
            in_=t32[:], in_offset=None,
            compute_op=mybir.AluOpType.add)
        # A fix: add neighbours' partials straight into the f16 atab rows
        nc.gpsimd.indirect_dma_start(
            out=atab[:],
            out_offset=bass.IndirectOffsetOnAxis(ap=afo32[:], axis=0),
            in_=t32[:], in_offset=None,
            compute_op=mybir.AluOpType.add)
        # reload the fixed means for the pooled-atom column sums
        at16 = const.tile([128, NCH * 128], F16)
        nc.sync.dma_start(at16[:].rearrange("p (c f) -> p c f", c=NCH),
                          atab[:].rearrange("(c p) f -> p c f", p=128))

        # ---- h feature sums via histogram/colsum matmuls ----
        tps0 = psz.tile([128, 1], F32, name="tps_xa", tag="tiny")
        for cch in range(NCH):
            nc.tensor.matmul(tps0[:],
                             mk16[:, D_OUT * cch:D_OUT * cch + N_AE],
                             cntv[:, cch:cch + 1],
                             start=(cch == 0), stop=(cch == NCH - 1))
        sxa16 = const.tile([128, 1], F16)
        nc.scalar.copy(sxa16[:], tps0[:])
        tps1 = psz.tile([128, 1], F32, name="tps_xe", tag="tiny")
        for cch in range(NCH):
            nc.tensor.matmul(tps1[:],
                             mk16[:, D_OUT * cch + N_AE:D_OUT * (cch + 1)],
                             cntv[:, cch:cch + 1],
                             start=(cch == 0), stop=(cch == NCH - 1))
        sxe16 = const.tile([128, 1], F16)
        nc.scalar.copy(sxe16[:], tps1[:])
        sxd = const.tile([128, 1], F32)
        nc.vector.tensor_reduce(out=sxd[:], in_=sums[:, 32:64],
                                axis=mybir.AxisListType.X,
                                op=mybir.AluOpType.add)
        sxd16 = const.tile([128, 1], F16)
        nc.scalar.copy(sxd16[:], sxd[:])
        tps = psz.tile([128, 1], F32, name="tps_a", tag="tiny")
        for cch in range(NCH):
            nc.tensor.matmul(tps[:], at16[:, 128 * cch:128 * (cch + 1)],
                             lcv[:, cch:cch + 1],
                             start=(cch == 0), stop=(cch == NCH - 1))
        sma16 = const.tile([128, 1], F16)
        nc.scalar.copy(sma16[:], tps[:])
        for c0 in range(0, NW, 8):
            cw = min(8, NW - c0)
            wsb = strm.tile([128, 8 * 128], F16, name="wsb", tag="wsb")
            nc.sync.dma_start(
                wsb[:, 0:cw * 128].rearrange("p (c f) -> p c f", c=cw),
                wtab[:].rearrange("(c p) f -> p c f", p=128)[:, c0:c0 + cw, :])
            if c0 == 0:
                tps2 = psz.tile([128, 1], F32, name="tps_e", tag="tiny")
            for cc in range(cw):
                nc.tensor.matmul(tps2[:], wsb[:, 128 * cc:128 * (cc + 1)],
                                 hev[:, c0 + cc:c0 + cc + 1],
                                 start=(c0 == 0 and cc == 0),
                                 stop=(c0 + cc == NW - 1))
        sme16 = const.tile([128, 1], F16)
        nc.scalar.copy(sme16[:], tps2[:])
        sp = const.tile([128, 2], F32)
        sxs = [sxa16, sxe16, sxd16]
        for m in range(2):
            tps3 = psz.tile([128, 1], F32, name="tps_p", tag="tiny")
            for k in range(3):
                nc.tensor.matmul(tps3[:],
                                 wx[:, D_OUT * k + 128 * m:D_OUT * k + 128 * (m + 1)],
                                 sxs[k][:],
                                 start=(k == 0), stop=True,
                                 skip_group_check=(k > 0))
            nc.tensor.matmul(tps3[:], wpa[:, 128 * m:128 * (m + 1)], sma16[:],
                             start=False, stop=True, skip_group_check=True)
            nc.tensor.matmul(tps3[:], wpe[:, 128 * m:128 * (m + 1)], sme16[:],
                             start=False, stop=True, skip_group_check=True)
            nc.scalar.copy(sp[:, m:m + 1], tps3[:])

        # ---- Phase C: pooled parts + fused add/stats ----
        zcts = [None] * ngrp

        def make_zct(g):
            awt = strm.tile([128, 128], F16, name="awt", tag="awt")
            nc.gpsimd.dma_gather(
                out_ap=awt[:].rearrange("p (a n) -> p a n", a=1),
                in_ap=atab[:],
                idxs_ap=awi[:, 8 * g:8 * (g + 1)],
                num_idxs=128, num_idxs_reg=128,
                elem_size=128, elem_step=128, transpose=True)
            zct = store.tile([128, D_OUT], F16, name=f"zct{g}", tag=f"zct{g}")
            for m in range(2):
                zp = psz.tile([128, 128], F32, name="zp", tag="zp")
                nc.tensor.matmul(zp[:], awt[:], wpa[:, 128 * m:128 * (m + 1)],
                                 start=True, stop=True)
                nc.scalar.copy(zct[:, 128 * m:128 * (m + 1)], zp[:])
            zcts[g] = zct

        for bt in range(ntile):
            for g in range(2 * bt // gsz, (2 * bt + 2 + gsz - 1) // gsz):
                if g < ngrp and zcts[g] is None:
                    make_zct(g)
            gat = strm.tile([128, 2 * BLK], F16, name="gat", tag="gat")
            nc.gpsimd.dma_gather(
                out_ap=gat[:].rearrange("p (a n) -> p a n", a=1),
                in_ap=wtab[:],
                idxs_ap=egi[:, 64 * bt:64 * (bt + 1)],
                num_idxs=2 * BLK, num_idxs_reg=2 * BLK,
                elem_size=128, elem_step=128, transpose=True,
                single_packet=False)
            for m in range(2):
                for j in range(2):
                    b = 2 * bt + j
                    g = b // gsz
                    pc = psm.tile([128, BLK], F32, name=f"pc{m}", tag=f"pp{m}")
                    nc.tensor.matmul(pc[:],
                                     wpe[:, 128 * m:128 * (m + 1)],
                                     gat[:, BLK * j:BLK * (j + 1)],
                                     start=True, stop=True)
                    nc.tensor.matmul(pc[:],
                                     zcts[g][:, 128 * m:128 * (m + 1)],
                                     ohtt[bt][:, BLK * j:BLK * (j + 1)],
                                     start=False, stop=True,
                                     skip_group_check=True)
                    hslice = hsb[m][:, BLK * b:BLK * (b + 1)]
                    nc.vector.tensor_tensor(out=hslice, in0=hslice, in1=pc[:],
                                            op=mybir.AluOpType.add)
                dump = strm.tile([128, 2 * BLK], F16, name="dump", tag="dump")
                hslice2 = hsb[m][:, 2 * BLK * bt:2 * BLK * (bt + 1)]
                nc.scalar.activation(
                    dump[:], hslice2,
                    mybir.ActivationFunctionType.Square, bias=0.0,
                    accum_out=sums[:, 2 * bt + m:2 * bt + m + 1])

        # ---- BN stats: local reduce, AllGather, combine, affine ----
        s4 = const.tile([128, 4], F32)
        sv2 = sums[:, 0:2 * ntile].rearrange("p (b m) -> p m b", m=2)
        for m in range(2):
            nc.vector.tensor_reduce(out=s4[:, 2 + m:2 + m + 1], in_=sv2[:, m, :],
                                    axis=mybir.AxisListType.X,
                                    op=mybir.AluOpType.add)
        nc.vector.tensor_scalar_add(s4[:, 0:2], sp[:], 0.0)
        nc.sync.dma_start(statin[:], s4[:])
        nc.gpsimd.collective_compute(
            "AllGather", mybir.AluOpType.bypass, replica_groups=groups,
            ins=[statin.opt()], outs=[statout.opt()])
        sg = const.tile([128, 4 * n_cores], F32)
        nc.sync.dma_start(sg[:].rearrange("p (c f) -> p c f", c=n_cores),
                          statout[:].rearrange("c p f -> p c f"))
        s4g = const.tile([128, 4], F32)
        nc.vector.tensor_reduce(out=s4g[:],
                                in_=sg[:].rearrange("p (c f) -> p f c", c=n_cores),
                                axis=mybir.AxisListType.X,
                                op=mybir.AluOpType.add)

        n_total = float(n_cores * rpc)
        mu4 = const.tile([128, 4], F32)
        nc.scalar.mul(mu4[:], s4g[:], 1.0 / n_total)
        mu, ex2 = mu4[:, 0:2], mu4[:, 2:4]
        mu2 = const.tile([128, 2], F32)
        nc.vector.tensor_tensor(out=mu2[:], in0=mu, in1=mu,
                                op=mybir.AluOpType.mult)
        var = const.tile([128, 2], F32)
        nc.vector.tensor_tensor(out=var[:], in0=ex2, in1=mu2[:],
                                op=mybir.AluOpType.subtract)
        vare = const.tile([128, 2], F32)
        nc.vector.tensor_scalar_add(vare[:], var[:], EPS)
        std = const.tile([128, 2], F32)
        nc.scalar.activation(std[:], vare[:],
                             mybir.ActivationFunctionType.Sqrt, bias=0.0)
        rstd = const.tile([128, 2], F32)
        nc.vector.reciprocal(rstd[:], std[:])
        a_t = const.tile([128, 2], F32)
        nc.vector.tensor_tensor(out=a_t[:], in0=gb[:, 0:2], in1=rstd[:],
                                op=mybir.AluOpType.mult)
        mua = const.tile([128, 2], F32)
        nc.vector.tensor_tensor(out=mua[:], in0=mu, in1=a_t[:],
                                op=mybir.AluOpType.mult)
        baff = const.tile([128, 2], F32)
        nc.vector.tensor_tensor(out=baff[:], in0=gb[:, 2:4], in1=mua[:],
                                op=mybir.AluOpType.subtract)

        # ---- P3: out = relu(h * a + b) in f16, streamed out ----
        SUP = 4 * BLK   # 2048-col supertiles
        outv = d_out.rearrange("(c p) n -> p c n", p=128)
        for st in range(rpc // SUP):
            for m in range(2):
                osb = pO.tile([128, SUP], F16, name="osb", tag="osb")
                hs = hsb[m][:, SUP * st:SUP * (st + 1)]
                if (2 * st + m) % 3 == 2:
                    nc.vector.tensor_scalar(
                        out=osb[:], in0=hs, scalar1=a_t[:, m:m + 1],
                        scalar2=baff[:, m:m + 1],
                        op0=mybir.AluOpType.mult, op1=mybir.AluOpType.add)
                    nc.vector.tensor_scalar_max(osb[:], osb[:], 0.0)
                else:
                    nc.scalar.activation(osb[:], hs,
                                         mybir.ActivationFunctionType.Relu,
                                         scale=a_t[:, m:m + 1],
                                         bias=baff[:, m:m + 1])
                nc.sync.dma_start(outv[:, m, SUP * st:SUP * (st + 1)], osb[:])

    nc.compile()
    return nc


def prep_shard(x16, xae16, atom_idx, ele_idx, r0, r1, w_blk, gsz, W,
               inv_cnt, counts_glob, firsts, lasts, pos_of_seg, c, n_cores,
               gS, LT):
    """Host-side shard prep for core c. Pure slicing/layout/index work."""
    rpc = r1 - r0
    nblk = rpc // BLK
    ngrp = nblk // gsz
    ntile = nblk // 2
    WROW = W + 8
    a = atom_idx[r0:r1]
    e = ele_idx[r0:r1]
    first, last = int(a[0]), int(a[-1])
    span = last - first + 1

    def pair(t3):
        """[nblk, 128, C] -> [ntile, 128, 2C] merging adjacent blocks."""
        C = t3.shape[2]
        return np.ascontiguousarray(
            t3.reshape(ntile, 2, 128, C).transpose(0, 2, 1, 3)
        ).reshape(ntile, 128, 2 * C)

    # x^T tiles: per block, feature-major [feat%128, k, row]
    xs16 = x16[r0:r1]
    xt = pair(np.ascontiguousarray(
        xs16.reshape(nblk, BLK, 3, 128).transpose(0, 3, 2, 1)
    ).reshape(nblk, 128, 3 * BLK))

    # p1 tiles: per block, 4 row-major x_ae tiles [128, 256]
    p1 = pair(np.ascontiguousarray(
        xae16[r0:r1].reshape(nblk, TPB, 128, 2 * N_AE)
        .transpose(0, 2, 1, 3)).reshape(nblk, 128, TPB * 2 * N_AE))
    gbase = np.array([int(a[BLK * gsz * g]) for g in range(ngrp)], np.int64)
    a_loc = (a - np.repeat(gbase, BLK * gsz)).astype(np.float32)
    # alv[p, 4b+t] = a_loc of row 512b+128t+p
    alv = np.ascontiguousarray(
        a_loc.reshape(nblk * TPB, 128).T).astype(np.float32)

    # onehot^T tiles: oht[b][s, r] = (a_loc[b*512+r] == s)
    oht = np.zeros((nblk, 128, BLK), dtype=np.float16)
    ai = a_loc.astype(np.int64).reshape(nblk, BLK)
    assert ai.max() < 128
    bb, rr = np.meshgrid(np.arange(nblk), np.arange(BLK), indexing="ij")
    oht[bb, ai, rr] = 1.0
    oht = pair(oht)

    # scatter offsets (relative to the group's private table) + scale
    scl = np.zeros((w_blk, ngrp), dtype=np.float32)
    sofs = np.zeros((w_blk, ngrp), dtype=np.int32)
    lanes = np.arange(w_blk)
    for g in range(ngrp):
        rows_g = a[BLK * gsz * g:BLK * gsz * (g + 1)]
        gspan = int(rows_g[-1]) - int(rows_g[0]) + 1
        assert gspan <= w_blk, f"group seg span {gspan} > w_blk {w_blk}"
        gb_rel = int(rows_g[0]) - first
        # lanes beyond the group span carry zeros; park them on the junk row
        sidx = np.where(lanes < gspan, gb_rel + lanes - gS[g], LT[g])
        assert sidx.min() >= 0
        assert np.where(lanes < gspan, sidx, 0).max() < LT[g]
        sofs[:, g] = sidx
        seg_glob = np.minimum(int(rows_g[0]) + lanes, NUM_SEG - 1)
        scl[:, g] = inv_cnt[seg_glob]

    # E-gather positions (into gtable), wrapped per 1024-idx batch
    pos = pos_of_seg[e]                     # [rpc]
    egi = np.concatenate(
        [_wrap16(pos[1024 * t:1024 * (t + 1)]) for t in range(rpc // 1024)],
        axis=1)                             # [128, rpc//16]

    # A-window gather indices per group (into atab, window-relative)
    awi_cols = []
    for g in range(ngrp):
        gb_rel = int(a[BLK * gsz * g]) - first
        idx = np.minimum(gb_rel + np.arange(128), W - 1)
        awi_cols.append(_wrap16(idx))
    awi = np.concatenate(awi_cols, axis=1)  # [128, 8*ngrp]

    NCH = (W + 127) // 128
    NW = n_cores * WROW // 128

    # mirror scatter for the last group: the last segment's lane lands on
    # mtab row 0, all other lanes on junk row 4
    last_gb_rel = int(a[BLK * gsz * (ngrp - 1)]) - first
    sof2 = np.full((w_blk, 1), 4, np.int32)
    sof2[(span - 1) - last_gb_rel, 0] = 0

    def shared(i):
        return lasts[i] == firsts[i + 1]

    # wtab pair fix (same on every core): row 4c+2 (E-first of core c) adds
    # into the left core's last row; 4c+3 (E-last) into the right core's row 0
    DUMW = W + 6
    wfo = np.full((32, 1), DUMW, np.int32)
    for cc in range(n_cores):
        if cc > 0 and shared(cc - 1):
            wfo[4 * cc + 2, 0] = (cc - 1) * WROW + (lasts[cc - 1] - firsts[cc - 1])
        if cc < n_cores - 1 and shared(cc):
            wfo[4 * cc + 3, 0] = (cc + 1) * WROW + 0
    # local A-half fix: neighbours' A partials into my first/last atab rows
    DUMF = NCH * 128 - 1
    assert span - 1 < DUMF
    afo32 = np.full((32, 1), DUMF, np.int32)
    if c > 0 and shared(c - 1):
        afo32[4 * (c - 1) + 1, 0] = 0
    if c < n_cores - 1 and shared(c):
        afo32[4 * (c + 1) + 0, 0] = span - 1

    # local per-window-row atom counts (for pooled-atom feature sums)
    lcv = np.zeros((128, 8), np.float16)
    lcnt = np.bincount((a - first).astype(np.int64), minlength=NCH * 128)
    lcv[:, 0:NCH] = lcnt[:NCH * 128].reshape(NCH, 128).T.astype(np.float16)
    # global counts per window row (recovers local x_ae sums from the
    # globally-scaled window table); zero beyond the span to mask junk rows
    cntv = np.zeros((128, 8), np.float16)
    gc = np.zeros(NCH * 128, np.int64)
    gc[:span] = counts_glob[first:first + span]
    cntv[:, 0:NCH] = gc.reshape(NCH, 128).T.astype(np.float16)
    # ele-position histogram over wtab rows (for pooled-ele feature sums)
    hev = (np.bincount(pos, minlength=NW * 128)[:NW * 128]
           .reshape(NW, 128).T.astype(np.float16))

    return {
        "xt": xt, "p1": p1, "alv": alv, "oht": oht, "egi": egi, "awi": awi,
        "scl": scl, "sof": sofs, "sof2": sof2, "wfo": wfo, "afo32": afo32,
        "lcv": lcv, "hev": hev, "cntv": cntv,
    }


def run(x, atom_idx, ele_idx, W1, b1, gamma, beta, n_cores=8, runner=None):
    x = np.asarray(x, dtype=np.float32)
    atom_idx = np.asarray(atom_idx).astype(np.int64)
    ele_idx = np.asarray(ele_idx).astype(np.int64)
    W1 = np.asarray(W1, dtype=np.float32)
    gamma = np.asarray(gamma, dtype=np.float32)
    beta = np.asarray(beta, dtype=np.float32)

    n = x.shape[0]
    assert n % n_cores == 0
    rpc = n // n_cores
    assert rpc % (2 * BLK) == 0
    assert np.all(np.diff(atom_idx) >= 0), "atom_idx must be sorted"

    counts = np.bincount(atom_idx, minlength=NUM_SEG).astype(np.int64)
    inv_cnt = (1.0 / np.maximum(counts, 1)).astype(np.float32)

    firsts = np.array([int(atom_idx[rpc * c]) for c in range(n_cores)], np.int64)
    lasts = np.array([int(atom_idx[rpc * (c + 1) - 1]) for c in range(n_cores)],
                     np.int64)
    spans = lasts - firsts + 1
    W = int(((spans.max() + 7) // 8) * 8)
    if W % 16 != 8:
        W += 8     # keep n_cores*(W+8) a multiple of 128
    WROW = W + 8

    # segment -> gtable row (owner core window position); gaps -> zero row
    pos_of_seg = np.full(NUM_SEG, W + 4, np.int64)
    for c in range(n_cores):
        segs = np.arange(firsts[c], lasts[c] + 1)
        pos_of_seg[segs] = c * WROW + (segs - firsts[c])

    # group size: largest g with all group spans <= 128
    def max_span(g):
        rows = BLK * g
        return max(int(atom_idx[min(i + rows, n) - 1]) - int(atom_idx[i]) + 1
                   for i in range(0, n, rows))
    gsz, w_blk = 1, None
    for g in (8, 4, 2, 1):
        if (rpc // BLK) % g:
            continue
        s = max_span(g)
        if s <= 120 or g == 1:
            gsz = g
            w_blk = min(128, max(8, ((s + 7) // 8) * 8))
            break
    assert w_blk is not None and max_span(gsz) <= w_blk, "segment span too large"

    # static per-group private-table geometry (uniform across cores):
    # 128-aligned window-row base gS[g] and table rows LT[g]
    ngrp_ = (rpc // BLK) // gsz
    gS, LT = [], []
    for g in range(ngrp_):
        lo, hi = 10 ** 9, 0
        for c in range(n_cores):
            i0 = rpc * c + BLK * gsz * g
            gb_rel = int(atom_idx[i0]) - int(atom_idx[rpc * c])
            lo = min(lo, gb_rel)
            hi = max(hi, gb_rel + w_blk)
        s = (lo // 128) * 128
        gS.append(s)
        LT.append(((hi - s + 127) // 128) * 128)

    x16 = x.astype(np.float16)
    xae16 = x16[:, :2 * N_AE]

    in_maps = []
    shared = {
        "iot": np.tile(np.arange(w_blk, dtype=np.float16), (128, 1)),
        "wx": np.ascontiguousarray(np.concatenate(
            [W1[0:128], W1[256:384], W1[512:640]], axis=0)
            .reshape(3, 128, D_OUT).transpose(1, 0, 2)
            .reshape(128, 3 * D_OUT)).astype(np.float16),
        "wpa": W1[128:256].astype(np.float16),
        "wpe": W1[384:512].astype(np.float16),
    }
    gbt = np.zeros((128, 4), dtype=np.float32)
    gbt[:, 0:2] = gamma.reshape(2, 128).T
    gbt[:, 2:4] = beta.reshape(2, 128).T
    shared["gb"] = gbt

    for c in range(n_cores):
        m = prep_shard(x16, xae16, atom_idx, ele_idx, rpc * c, rpc * (c + 1),
                       w_blk, gsz, W, inv_cnt, counts, firsts, lasts,
                       pos_of_seg, c, n_cores, gS, LT)
        m.update(shared)
        in_maps.append(m)

    nc = build_program(n_cores, rpc, w_blk, gsz, W, gS, LT)
    global LAST_NC
    LAST_NC = nc
    if runner is None:
        res = run_bass_kernel_spmd(nc, in_maps, core_ids=list(range(n_cores)))
        outs = [res.results[c]["out"] for c in range(n_cores)]
    else:
        outs = runner(nc, in_maps)

    full = np.concatenate(outs, axis=1)                    # [256, n] f16
    return np.ascontiguousarray(full.T).astype(np.float32)


def kernel(**inputs):
    return run(inputs["x"], inputs["atom_idx"], inputs["ele_idx"],
               inputs["W1"], inputs["b1"], inputs["gamma"], inputs["beta"])
